# revision 1
# baseline (speedup 1.0000x reference)
"""MinkowskiConvolution forward on 8 TRN2 NeuronCores.

Computation (reference):
    out[n, o] = sum_k sum_c features[idx[k, n], c] * W[k, c, o]
with idx[k, n] == -1 meaning "no neighbor" (contributes zero).

Strategy:
  - Shard output points across the 8 cores (37504 padded points each);
    replicate the feature table (with an appended zero row) and the small
    kernel tensor. No collectives needed.
  - Host prep: remap idx -1 -> zero row, transpose idx to point-major,
    cast features/kernel to bf16, stack the 27 per-offset weight matrices
    (+1 zero pad) into 7 groups of 4 so each group's matmul contracts over
    4*32 = 128 channels.
  - Device, per 128-point tile:
      * 26 indirect DMAs (one per non-center offset) gather 128 rows each
        from the table in HBM: dest [128, 32] with one index per partition
        (the only indirect-DMA shape the TRN2 DGE unrolls correctly; it
        costs ~1.4us/instruction, which dominates the kernel).
      * the center offset is the identity map, so it is a dense DMA.
      * PE transposes the gathered [points, channels] blocks to
        [channels, points] via identity matmuls (bf16 PSUM), DVE copies
        them back to SBUF, and 7 stacked matmuls accumulate [128, 64] f32
        in PSUM; ACT copies out, HWDGE streams results to DRAM.
"""

import os
import sys
from contextlib import ExitStack

import numpy as np

sys.path.insert(0, os.path.dirname(os.path.abspath(__file__)))

import ml_dtypes

import concourse.bass as bass
import concourse.bacc as bacc
import concourse.mybir as mybir
import concourse.tile as tile
from concourse.bass_utils import run_bass_kernel_spmd
from concourse.masks import make_identity

P = 128
N = 300_000
K = 27
CENTER = K // 2
KPAD = 28          # 27 offsets + 1 zero-weight pad -> 7 groups of 4
NGROUPS = 7
INC = 32
OUTC = 64
NCORES = 8
NPAD = 300_032     # 8 * 37504
NP_CORE = NPAD // NCORES          # 37504
NTILES = NP_CORE // P             # 293
R = NPAD + 1                       # table rows + zero row (300033)
ZROW = NPAD

_BF16 = mybir.dt.bfloat16
_F32 = mybir.dt.float32
_I32 = mybir.dt.int32


def build_nc(ntiles=NTILES, r=R, core_row0=0, center_static=True):
    """Build + compile the per-core Bass program.

    core_row0: not needed — the center offset's rows are the shard's own
    rows; each core gets its own `row0` scalar via the idx input instead.
    To keep one program for all cores, the center rows are located via a
    dedicated `crow` input tensor holding the shard's global row offset
    baked into the DMA source by... simplest: the center DMA reads from a
    per-core `cfeat` DRAM input [ntiles*P, INC] (the shard's own feature
    rows, prepared on host).
    """
    nc = bacc.Bacc("TRN2", target_bir_lowering=False, debug=False)
    np_core = ntiles * P
    table = nc.dram_tensor("table", [r, INC], _BF16, kind="ExternalInput")
    idxT = nc.dram_tensor("idx", [np_core, K], _I32, kind="ExternalInput")
    cfeat = nc.dram_tensor("cfeat", [np_core, INC], _BF16, kind="ExternalInput")
    wst = nc.dram_tensor("wst", [P, NGROUPS * OUTC], _BF16, kind="ExternalInput")
    out = nc.dram_tensor("out", [np_core, OUTC], _F32, kind="ExternalOutput")

    with ExitStack() as ctx:
        tc = ctx.enter_context(tile.TileContext(nc))
        const = ctx.enter_context(tc.tile_pool(name="const", bufs=1))
        w_sb = const.tile([P, NGROUPS * OUTC], _BF16)
        nc.sync.dma_start(out=w_sb[:], in_=wst[:])
        ident = const.tile([P, P], _BF16)
        make_identity(nc, ident[:])

        idxp = ctx.enter_context(tc.tile_pool(name="idxp", bufs=4))
        gp = ctx.enter_context(tc.tile_pool(name="gp", bufs=4))
        gtp = ctx.enter_context(tc.tile_pool(name="gtp", bufs=3))
        osb = ctx.enter_context(tc.tile_pool(name="osb", bufs=4))
        pa = ctx.enter_context(tc.tile_pool(name="pa", bufs=2, space="PSUM"))
        pb = ctx.enter_context(tc.tile_pool(name="pb", bufs=2, space="PSUM"))
        po = ctx.enter_context(tc.tile_pool(name="po", bufs=2, space="PSUM"))

        for t in range(ntiles):
            idx_tile = idxp.tile([P, K], _I32, tag="idx")
            nc.sync.dma_start(out=idx_tile[:], in_=idxT[t * P:(t + 1) * P, :])
            g = gp.tile([P, KPAD * INC], _BF16, tag="g")
            for k in range(K):
                if center_static and k == CENTER:
                    nc.sync.dma_start(
                        out=g[:, k * INC:(k + 1) * INC],
                        in_=cfeat[t * P:(t + 1) * P, :],
                    )
                    continue
                nc.gpsimd.indirect_dma_start(
                    out=g[:, k * INC:(k + 1) * INC],
                    out_offset=None,
                    in_=table[:],
                    in_offset=bass.IndirectOffsetOnAxis(
                        ap=idx_tile[:, k:k + 1], axis=0
                    ),
                )
            # zero the 28th (pad) offset lane so group 6 contracts cleanly
            nc.vector.memset(g[:, K * INC:], 0.0)

            ps_a = pa.tile([P, 4 * P], _BF16, tag="pa")
            ps_b = pb.tile([P, 3 * P], _BF16, tag="pb")
            for gi in range(NGROUPS):
                dst = (
                    ps_a[:, gi * P:(gi + 1) * P]
                    if gi < 4
                    else ps_b[:, (gi - 4) * P:(gi - 3) * P]
                )
                nc.tensor.transpose(dst, g[:, gi * P:(gi + 1) * P], ident[:])
            gt = gtp.tile([P, KPAD * INC], _BF16, tag="gt")
            nc.vector.tensor_copy(out=gt[:, 0:4 * P], in_=ps_a[:])
            nc.vector.tensor_copy(out=gt[:, 4 * P:7 * P], in_=ps_b[:])
            ps_o = po.tile([P, OUTC], _F32, tag="po")
            for gi in range(NGROUPS):
                nc.tensor.matmul(
                    ps_o[:],
                    gt[:, gi * P:(gi + 1) * P],
                    w_sb[:, gi * OUTC:(gi + 1) * OUTC],
                    start=(gi == 0),
                    stop=(gi == NGROUPS - 1),
                )
            ot = osb.tile([P, OUTC], _F32, tag="ot")
            nc.scalar.copy(out=ot[:], in_=ps_o[:])
            nc.sync.dma_start(out=out[t * P:(t + 1) * P, :], in_=ot[:])
    nc.compile()
    return nc


def prep_inputs(features, kernel, neighbor_idx, npad=NPAD, r=R, zrow=ZROW):
    """Host-side prep: bf16 table with zero row, stacked weights, safe idx."""
    n = features.shape[0]
    table = np.zeros((r, INC), dtype=ml_dtypes.bfloat16)
    table[:n] = features.astype(ml_dtypes.bfloat16)

    wst = np.zeros((P, NGROUPS * OUTC), dtype=ml_dtypes.bfloat16)
    kb = kernel.astype(ml_dtypes.bfloat16)
    for k in range(K):
        g, a = divmod(k, 4)
        wst[a * INC:(a + 1) * INC, g * OUTC:(g + 1) * OUTC] = kb[k]

    idx_safe = np.full((K, npad), zrow, dtype=np.int32)
    idx_safe[:, :neighbor_idx.shape[1]] = np.where(
        neighbor_idx < 0, zrow, neighbor_idx
    )
    idx_t = np.ascontiguousarray(idx_safe.T)  # [npad, K] point-major
    return table, wst, idx_t


_nc_cache = {}


def kernel(features, kernel, neighbor_idx):
    center_static = bool(
        np.array_equal(
            neighbor_idx[CENTER], np.arange(neighbor_idx.shape[1], dtype=np.int32)
        )
    )
    key = ("full", center_static)
    if key not in _nc_cache:
        _nc_cache[key] = build_nc(center_static=center_static)
    nc = _nc_cache[key]

    table, wst, idx_t = prep_inputs(features, kernel, neighbor_idx)
    in_maps = []
    for ci in range(NCORES):
        lo = ci * NP_CORE
        in_maps.append(
            {
                "table": table,
                "wst": wst,
                "idx": idx_t[lo:lo + NP_CORE],
                "cfeat": np.ascontiguousarray(table[lo:lo + NP_CORE]),
            }
        )
    res = run_bass_kernel_spmd(nc, in_maps, core_ids=list(range(NCORES)))
    out = np.concatenate([res.results[ci]["out"] for ci in range(NCORES)], axis=0)
    return np.ascontiguousarray(out[:N])


if __name__ == "__main__":
    rng = np.random.default_rng(1)
    f = rng.standard_normal((N, INC), dtype=np.float32)
    w = rng.standard_normal((K, INC, OUTC), dtype=np.float32) * 0.03
    idx = rng.integers(-1, N, size=(K, N)).astype(np.int32)
    idx[CENTER] = np.arange(N, dtype=np.int32)
    o = kernel(f, w, idx)
    print("out", o.shape, o.dtype, float(np.abs(o).mean()))

